# revision 1
# baseline (speedup 1.0000x reference)
"""Bass/Trainium2 kernel for nn_LinearLayerAttention (linear attention rank-1
state update). Self-contained: accepts FULL inputs, shards batch across 8
NeuronCores, returns FULL outputs (out, s_new, z_new).

Math (per batch b):
  y[c]   = mean_{h,w} x[b,c,h,w]
  q      = elu(conv1d_same(y, wq)) + 1          (= relu(a) + exp(min(a,0)))
  k      = elu(conv1d_same(y, wk)) + 1
  V      = depthwise3x3_same(x[b], wv)          flattened to V_flat[d], d=(c',h,w)
  s_new[c,d] = s[c,d] + k[c] * V_flat[d]
  z_new  = z + k
  QZ     = 1 / sum_c q[c]*(z_new[c] + eps)
  out[d] = QZ * sum_c q[c]*s_new[c,d]
         = sum_c (QZ*q[c])*s[c,d] + (QZ*sum_c q[c]k[c]) * V_flat[d]

The dominant cost is streaming s (51.4 MB) in and s_new (51.4 MB) out per
core -> memory bound. Everything is computed in a single pass over s with
one fused DVE op per stream (scalar_tensor_tensor).
"""

import numpy as np

import concourse.bass as bass
import concourse.mybir as mybir
from concourse import bacc
from concourse.tile import TileContext

B = 8
C = 64
H = 56
W = 56
HW = H * W              # 3136
D = C * HW              # 200704
P = 128                 # SBUF partitions used for the streaming tiles
F = D // P              # 1568 f32 per partition per s-row
RPC = 2                 # s rows per DMA chunk (2 rows = 1.6 MB per dma_start)
PADW = W + 2            # 58
EPS = 1e-6
FP = mybir.dt.float32

_CACHE: dict = {}


def _build_module():
    mult = mybir.AluOpType.mult
    add = mybir.AluOpType.add
    AX = mybir.AxisListType

    nc = bacc.Bacc("TRN2", target_bir_lowering=False, debug=False)

    x_d = nc.dram_tensor("x", [C, HW], FP, kind="ExternalInput")
    s_d = nc.dram_tensor("s", [C, D], FP, kind="ExternalInput")
    z_d = nc.dram_tensor("z", [1, C], FP, kind="ExternalInput")
    wq_d = nc.dram_tensor("wq", [1, 3], FP, kind="ExternalInput")
    wk_d = nc.dram_tensor("wk", [1, 3], FP, kind="ExternalInput")
    wv_d = nc.dram_tensor("wv", [C, 9], FP, kind="ExternalInput")
    out_d = nc.dram_tensor("out", [C, HW], FP, kind="ExternalOutput")
    snew_d = nc.dram_tensor("s_new", [C, D], FP, kind="ExternalOutput")
    znew_d = nc.dram_tensor("z_new", [1, C], FP, kind="ExternalOutput")

    # HBM views for the streaming tiles: s row c as [128, 1568]
    s_v = s_d.ap().rearrange("c (p f) -> p c f", p=P)        # [128, 64, 1568]
    snew_v = snew_d.ap().rearrange("c (p f) -> p c f", p=P)  # [128, 64, 1568]
    out_v = out_d.ap().rearrange("c (g f) -> (c g) f", g=P // C)  # [128, 1568]

    with TileContext(nc) as tc:
        with (
            tc.tile_pool(name="pers", bufs=1) as pers,
            tc.tile_pool(name="psum", bufs=1, space="PSUM") as psum,
            tc.tile_pool(name="sin", bufs=6) as sin,
            tc.tile_pool(name="sout", bufs=4) as sout,
        ):
            # ---------------- prologue: small tensors ----------------
            x_pad = pers.tile([C, PADW * PADW], FP)
            x_pad3 = x_pad.rearrange("c (a b) -> c a b", a=PADW)
            wv_sb = pers.tile([C, 9], FP)
            wq_sb = pers.tile([1, 3], FP)
            wk_sb = pers.tile([1, 3], FP)
            z_row = pers.tile([1, C], FP)
            ident = pers.tile([C, C], FP)

            # small loads on the SWDGE (gpsimd) queue so they don't queue
            # behind the big streaming loads on the sync HWDGE ring
            nc.vector.memset(x_pad[:], 0.0)
            nc.gpsimd.dma_start(
                out=x_pad3[:, 1 : H + 1, 1 : W + 1],
                in_=x_d.ap().rearrange("c (h w) -> c h w", h=H),
            )
            nc.gpsimd.dma_start(out=wv_sb[:], in_=wv_d.ap())
            nc.gpsimd.dma_start(out=wq_sb[:], in_=wq_d.ap())
            nc.gpsimd.dma_start(out=wk_sb[:], in_=wk_d.ap())
            nc.gpsimd.dma_start(out=z_row[:], in_=z_d.ap())

            from concourse.masks import make_identity

            make_identity(nc, ident[:])

            # y = mean over (h, w), computed from the padded buffer interior
            y_col = pers.tile([C, 1], FP)
            nc.vector.tensor_reduce(
                out=y_col[:], in_=x_pad3[:, 1 : H + 1, 1 : W + 1],
                axis=AX.XY, op=add,
            )
            nc.vector.tensor_scalar_mul(y_col[:], y_col[:], 1.0 / HW)

            # transpose y to a row via the PE: psum_row = y_col.T
            y_row_ps = psum.tile([1, C], FP)
            nc.tensor.transpose(y_row_ps[:], y_col[:], ident[:])
            y_row = pers.tile([1, C], FP)
            nc.scalar.copy(y_row[:], y_row_ps[:])

            # 3-tap conv1d along c with zero 'same' padding + phi (elu+1)
            def conv_phi(w_sb, name):
                pre = pers.tile([1, C], FP, name=f"{name}_pre")
                nc.vector.tensor_scalar_mul(pre[:], y_row[:], w_sb[0:1, 1:2])
                nc.vector.scalar_tensor_tensor(
                    out=pre[0:1, 1:C], in0=y_row[0:1, 0 : C - 1],
                    scalar=w_sb[0:1, 0:1], in1=pre[0:1, 1:C], op0=mult, op1=add,
                )
                nc.vector.scalar_tensor_tensor(
                    out=pre[0:1, 0 : C - 1], in0=y_row[0:1, 1:C],
                    scalar=w_sb[0:1, 2:3], in1=pre[0:1, 0 : C - 1],
                    op0=mult, op1=add,
                )
                # phi(a) = relu(a) + exp(min(a, 0))
                mn = pers.tile([1, C], FP, name=f"{name}_mn")
                nc.vector.tensor_scalar_min(mn[:], pre[:], 0.0)
                ex = pers.tile([1, C], FP, name=f"{name}_ex")
                nc.scalar.activation(ex[:], mn[:], mybir.ActivationFunctionType.Exp)
                rl = pers.tile([1, C], FP, name=f"{name}_rl")
                nc.vector.tensor_scalar_max(rl[:], pre[:], 0.0)
                ph = pers.tile([1, C], FP, name=f"{name}_ph")
                nc.vector.tensor_add(out=ph[:], in0=ex[:], in1=rl[:])
                return ph

            q_row = conv_phi(wq_sb, "q")
            k_row = conv_phi(wk_sb, "k")

            # z_new = z + k  (and store it)
            zn_row = pers.tile([1, C], FP)
            nc.vector.tensor_add(out=zn_row[:], in0=z_row[:], in1=k_row[:])
            nc.gpsimd.dma_start(out=znew_d.ap(), in_=zn_row[:])

            # QZ = 1 / sum(q * (z_new + eps)); qp = QZ*q; qzqk = QZ*sum(qp*k)
            t0 = pers.tile([1, C], FP)
            nc.vector.tensor_scalar_add(t0[:], zn_row[:], EPS)
            nc.vector.tensor_mul(out=t0[:], in0=t0[:], in1=q_row[:])
            qz_pre = pers.tile([1, 1], FP)
            nc.vector.tensor_reduce(out=qz_pre[:], in_=t0[:], axis=AX.X, op=add)
            qz = pers.tile([1, 1], FP)
            nc.vector.reciprocal(qz[:], qz_pre[:])
            qp_row = pers.tile([1, C], FP)
            nc.vector.tensor_scalar_mul(qp_row[:], q_row[:], qz[0:1, 0:1])
            t1 = pers.tile([1, C], FP)
            nc.vector.tensor_mul(out=t1[:], in0=qp_row[:], in1=k_row[:])
            qzqk = pers.tile([1, 1], FP)
            nc.vector.tensor_reduce(out=qzqk[:], in_=t1[:], axis=AX.X, op=add)

            # pack [qp | k | qzqk] into one row and broadcast to 128 partitions
            brow = pers.tile([1, 2 * C + 1], FP)
            nc.vector.tensor_copy(out=brow[0:1, 0:C], in_=qp_row[:])
            nc.vector.tensor_copy(out=brow[0:1, C : 2 * C], in_=k_row[:])
            nc.vector.tensor_copy(out=brow[0:1, 2 * C : 2 * C + 1], in_=qzqk[:])
            bc = pers.tile([P, 2 * C + 1], FP)
            nc.gpsimd.partition_broadcast(bc[:], brow[0:1, :])
            qp_b = bc[:, 0:C]          # [128, 64] column c = QZ*q[c]
            k_b = bc[:, C : 2 * C]     # [128, 64] column c = k[c]
            qzqk_b = bc[:, 2 * C : 2 * C + 1]  # [128, 1]

            # ---------------- V = depthwise 3x3 conv ----------------
            V = pers.tile([C, H, W], FP)
            nc.vector.tensor_scalar_mul(
                V[:], x_pad3[:, 0:H, 0:W], wv_sb[:, 0:1]
            )
            for t in range(1, 9):
                di, dj = divmod(t, 3)
                nc.vector.scalar_tensor_tensor(
                    out=V[:], in0=x_pad3[:, di : di + H, dj : dj + W],
                    scalar=wv_sb[:, t : t + 1], in1=V[:], op0=mult, op1=add,
                )
            # reshape V [64, 3136] -> V2 [128, 1568] (element order preserved)
            V2 = pers.tile([P, F], FP)
            nc.gpsimd.dma_start(out=V2[:], in_=V.rearrange("c a b -> c (a b)"))

            # ---------------- main stream over s ----------------
            acc = pers.tile([P, F], FP)
            for c0 in range(0, C, RPC):
                s_t = sin.tile([P, RPC, F], FP)
                nc.sync.dma_start(out=s_t[:], in_=s_v[:, c0 : c0 + RPC, :])
                sn_t = sout.tile([P, RPC, F], FP)
                for j in range(RPC):
                    c = c0 + j
                    # s_new = s + k[c]*V
                    nc.vector.scalar_tensor_tensor(
                        out=sn_t[:, j], in0=V2[:], scalar=k_b[:, c : c + 1],
                        in1=s_t[:, j], op0=mult, op1=add,
                    )
                    # acc += (QZ*q[c]) * s
                    if c == 0:
                        nc.vector.tensor_scalar_mul(
                            acc[:], s_t[:, j], qp_b[:, 0:1]
                        )
                    else:
                        nc.vector.scalar_tensor_tensor(
                            out=acc[:], in0=s_t[:, j],
                            scalar=qp_b[:, c : c + 1], in1=acc[:],
                            op0=mult, op1=add,
                        )
                nc.scalar.dma_start(out=snew_v[:, c0 : c0 + RPC, :], in_=sn_t[:])

            # ---------------- epilogue ----------------
            out_sb = pers.tile([P, F], FP)
            nc.vector.scalar_tensor_tensor(
                out=out_sb[:], in0=V2[:], scalar=qzqk_b[:, 0:1], in1=acc[:],
                op0=mult, op1=add,
            )
            nc.scalar.dma_start(out=out_v, in_=out_sb[:])

    nc.compile()
    return nc


def get_module():
    if "nc" not in _CACHE:
        _CACHE["nc"] = _build_module()
    return _CACHE["nc"]


def make_in_maps(x, s, z, wq, wk, wv):
    x = np.ascontiguousarray(np.asarray(x, dtype=np.float32))
    s = np.ascontiguousarray(np.asarray(s, dtype=np.float32))
    z = np.ascontiguousarray(np.asarray(z, dtype=np.float32))
    wq = np.ascontiguousarray(np.asarray(wq, dtype=np.float32)).reshape(1, 3)
    wk = np.ascontiguousarray(np.asarray(wk, dtype=np.float32)).reshape(1, 3)
    wv = np.ascontiguousarray(np.asarray(wv, dtype=np.float32)).reshape(C, 9)
    in_maps = []
    for b in range(B):
        in_maps.append(
            {
                "x": np.ascontiguousarray(x[b].reshape(C, HW)),
                "s": s[b],
                "z": z[b].reshape(1, C),
                "wq": wq,
                "wk": wk,
                "wv": wv,
            }
        )
    return in_maps


def assemble_outputs(results):
    out = np.stack([results[b]["out"] for b in range(B)]).reshape(B, C, H, W)
    s_new = np.stack([results[b]["s_new"] for b in range(B)])
    z_new = np.stack([results[b]["z_new"] for b in range(B)])
    return out, s_new, z_new


def kernel(x, s, z, wq, wk, wv):
    from concourse.bass_utils import run_bass_kernel_spmd

    nc = get_module()
    in_maps = make_in_maps(x, s, z, wq, wk, wv)
    res = run_bass_kernel_spmd(nc, in_maps, core_ids=list(range(B)))
    return assemble_outputs(res.results)
